# revision 21
# baseline (speedup 1.0000x reference)
"""MoE FFN (dense routing) Trainium2 kernel — expert-parallel over 8 NeuronCores.

Reference math (T=2048 tokens, D=1024, H=4096, E=8 experts, all dense):
    logits = x @ router_w + router_b          [T, E]
    probs  = softmax(logits)                  [T, E]
    h_e    = gelu(x @ W1[e] + b1[e])          [T, H]   (exact erf gelu)
    out_e  = h_e @ W2[e] + b2[e]              [T, D]
    y      = sum_e probs[:, e] * out_e        [T, D]
    returns (y, probs)

Sharding: expert parallel — core c owns expert e=c, computes the full dense
token set through its expert, scales by probs[:, e] on-chip, and the partial
outputs are summed on the host (the "all-reduce of the probability-weighted
combine"). Every core computes the router; core 0's probs are returned.
fc2_bias (zeros in this problem) is folded in on the host as probs @ fc2_bias.

Host-side input prep (free, not on the HW critical path): x is laid out as
x_T [D, T] bf16 (fc1 moving operand + router); W1 is packed [P, HK, DT, P]
bf16 for single-descriptor-per-partition DMA; W2 cast to bf16.

On-chip dataflow per core:
    fc1: h_T[hk] [128,512] = W1_chunk.T @ x_Tb  (bf16, PSUM fp32 accum)
         gelu+bias fused on ScalarE -> h bf16
    router (after fc1 of block 0): logits_T [8,T] via bf16 matmuls
         (N=512, full PE rate), PE-transposed back to [T,8] tiles,
         softmax on ACT/DVE, expert column via one-hot dot
    fc2: y[mt,dc] [128,512] accumulates over 32 H-chunks in 8 PSUM banks
         epilogue: ScalarE copy with per-partition scale = probs[:, e]
"""

import os
import sys

import numpy as np

for _p in ("/root/.axon_site/_ro/trn_rl_repo", "/opt/trn_rl_repo"):
    if os.path.isdir(_p) and _p not in sys.path:
        sys.path.append(_p)

B, S, D, H, E = 2, 1024, 1024, 4096, 8
T = B * S          # 2048 tokens
NCORES = 8
P = 128            # partitions
TT = T // P        # 16 token tiles of 128
NTB = 4            # token blocks for the FFN phase
TBLK = T // NTB    # 512 tokens per block
DT = D // P        # 8 contraction chunks for fc1 / router
HK = H // P        # 32 H-chunks
NMT = TBLK // P    # 4 M-subtiles per block
NDC = D // 512     # 2 N-subtiles of 512 per block

_CACHE = {}


def _build_nc():
    import concourse.bass as bass  # noqa: F401
    import concourse.tile as tile
    from concourse import bacc, mybir
    from concourse.masks import make_identity

    f32 = mybir.dt.float32
    bf16 = mybir.dt.bfloat16
    AF = mybir.ActivationFunctionType
    OP = mybir.AluOpType
    AX = mybir.AxisListType

    nc = bacc.Bacc(
        "TRN2",
        target_bir_lowering=False,
        debug=False,
        enable_asserts=False,
        num_devices=NCORES,
    )

    xtb_d = nc.dram_tensor("xtb", [D, T], bf16, kind="ExternalInput").ap()
    rw_d = nc.dram_tensor("rw", [D, E], bf16, kind="ExternalInput").ap()
    rbb_d = nc.dram_tensor("rbb", [P, E], f32, kind="ExternalInput").ap()
    sel_d = nc.dram_tensor("sel", [P, E], f32, kind="ExternalInput").ap()
    w1_d = nc.dram_tensor("w1p", [P, HK, DT, P], bf16, kind="ExternalInput").ap()
    b1_d = nc.dram_tensor("b1", [H], f32, kind="ExternalInput").ap()
    w2_d = nc.dram_tensor("w2", [H, D], bf16, kind="ExternalInput").ap()
    y_d = nc.dram_tensor("yp", [T, D], f32, kind="ExternalOutput").ap()
    probs_d = nc.dram_tensor("probs", [T, E], f32, kind="ExternalOutput").ap()

    with tile.TileContext(nc) as tc:
        from contextlib import ExitStack

        ctx = ExitStack()
        with ctx:
            consts = ctx.enter_context(tc.tile_pool(name="consts", bufs=1))
            xtbp = ctx.enter_context(tc.tile_pool(name="xtbp", bufs=1))
            w1p = ctx.enter_context(tc.tile_pool(name="w1p", bufs=5))
            w2p = ctx.enter_context(tc.tile_pool(name="w2p", bufs=5))
            hp = ctx.enter_context(tc.tile_pool(name="hp", bufs=HK))
            yp = ctx.enter_context(tc.tile_pool(name="yp", bufs=6))
            sm = ctx.enter_context(tc.tile_pool(name="sm", bufs=3))
            psp = ctx.enter_context(tc.tile_pool(name="psp", bufs=8, space="PSUM"))

            # constants on the SWDGE (gpsimd) queue so they don't head-block
            # the bulk sync-queue streams
            ident = consts.tile([P, P], f32, tag="ident", name="ident")
            make_identity(nc, ident)
            rw_sb = consts.tile([P, DT, E], bf16, tag="rw", name="rw_sb")
            nc.gpsimd.dma_start(rw_sb, rw_d.rearrange("(o p) e -> p o e", p=P))
            rbb_sb = consts.tile([P, E], f32, tag="rbb", name="rbb_sb")
            nc.gpsimd.dma_start(rbb_sb, rbb_d)
            sel_sb = consts.tile([P, E], f32, tag="sel", name="sel_sb")
            nc.gpsimd.dma_start(sel_sb, sel_d)
            b1_sb = consts.tile([P, HK], f32, tag="b1", name="b1_sb")
            nc.gpsimd.dma_start(b1_sb, b1_d.rearrange("(o p) -> p o", p=P))
            p_sb = consts.tile([P, TT], f32, tag="psb", name="p_sb")
            probs_all = consts.tile([P, TT, E], f32, tag="probs", name="probs_all")

            # x_T bf16 as one [P, DT, T] tile (p,dt,t) = x_T[dt*128+p, t];
            # block-0 columns DMA'd first so fc1(0) starts immediately, the
            # rest before the router (which reads all of T)
            xTb = xtbp.tile([P, DT, T], bf16, name="xTb")
            xtb_r = xtb_d.rearrange("(dt p) t -> p dt t", p=P)

            def load_xtb(tb):
                cs = slice(tb * TBLK, (tb + 1) * TBLK)
                nc.sync.dma_start(xTb[:, :, cs], xtb_r[:, :, cs])
            w2_r = w2_d.rearrange("(o p) d -> p o d", p=P)

            def fc1_block(tb):
                cs = slice(tb * TBLK, (tb + 1) * TBLK)
                h_tiles = []
                for hk4 in range(HK // 4):
                    w1_t = w1p.tile([P, 4, DT, P], bf16, name="w1_t")
                    nc.sync.dma_start(w1_t, w1_d[:, 4 * hk4 : 4 * hk4 + 4])
                    for j in range(4):
                        hk = 4 * hk4 + j
                        ps_h = psp.tile([P, TBLK], f32, tag="ps", name="ps_h")
                        for dt in range(DT):
                            nc.tensor.matmul(
                                ps_h,
                                lhsT=w1_t[:, j, dt, :],
                                rhs=xTb[:, dt, cs],
                                start=(dt == 0),
                                stop=(dt == DT - 1),
                            )
                        h_t = hp.tile([P, TBLK], bf16, name="h_t")
                        nc.scalar.activation(
                            h_t, ps_h, AF.Gelu, bias=b1_sb[:, hk : hk + 1]
                        )
                        h_tiles.append(h_t)
                return h_tiles

            def fc2_block(tb, h_tiles):
                ps_y = [
                    psp.tile([P, 512], f32, tag="ps", name=f"ps_y{g}")
                    for g in range(NMT * NDC)
                ]
                for hk4 in range(HK // 4):
                    w2_t = w2p.tile([P, 4, D], bf16, name="w2_t")
                    nc.sync.dma_start(w2_t, w2_r[:, 4 * hk4 : 4 * hk4 + 4])
                    for j in range(4):
                        hk = 4 * hk4 + j
                        for mt in range(NMT):
                            for dc in range(NDC):
                                nc.tensor.matmul(
                                    ps_y[mt * NDC + dc],
                                    lhsT=h_tiles[hk][:, mt * P : (mt + 1) * P],
                                    rhs=w2_t[:, j, dc * 512 : (dc + 1) * 512],
                                    start=(hk == 0),
                                    stop=(hk == HK - 1),
                                )
                for mt in range(NMT):
                    y_t = yp.tile([P, D], f32, name="y_t")
                    p_ap = p_sb[:, tb * NMT + mt : tb * NMT + mt + 1]
                    nc.scalar.activation(
                        y_t[:, 0:512], ps_y[mt * NDC], AF.Copy, scale=p_ap
                    )
                    nc.vector.tensor_scalar_mul(
                        y_t[:, 512:1024], ps_y[mt * NDC + 1], p_ap
                    )
                    r0 = tb * TBLK + mt * P
                    nc.sync.dma_start(y_d[r0 : r0 + P, :], y_t)

            def router():
                # logits_T [E, T] via bf16 matmuls at full PE rate (N=512).
                # All 4 matmul groups run back-to-back on PE (DVE copies
                # drain behind them), then all transposes: avoids PE<->DVE
                # round-trips per chunk.
                lt_ts = []
                for tc4 in range(NTB):
                    cs = slice(tc4 * TBLK, (tc4 + 1) * TBLK)
                    ps_lt = psp.tile([E, TBLK], f32, tag="ps", name="ps_lt")
                    for dt in range(DT):
                        nc.tensor.matmul(
                            ps_lt,
                            lhsT=rw_sb[:, dt, :],
                            rhs=xTb[:, dt, cs],
                            start=(dt == 0),
                            stop=(dt == DT - 1),
                        )
                    lt_t = sm.tile([E, TBLK], f32, tag="lt", name="lt_t", bufs=4)
                    nc.vector.tensor_copy(lt_t, ps_lt)
                    lt_ts.append(lt_t)
                for tc4 in range(NTB):
                    ps_lg = psp.tile([P, TBLK // P, E], f32, tag="ps", name="ps_lg")
                    for k in range(TBLK // P):
                        nc.tensor.transpose(
                            ps_lg[:, k, :],
                            lt_ts[tc4][:, k * P : (k + 1) * P],
                            ident[:E, :E],
                        )
                    logits4 = sm.tile([P, TBLK // P, E], f32, tag="lg4", name="logits4")
                    nc.vector.tensor_tensor(
                        logits4, ps_lg, rbb_sb[:, None, :].to_broadcast(
                            (P, TBLK // P, E)), OP.add
                    )
                    for k in range(TBLK // P):
                        tt = tc4 * (TBLK // P) + k
                        logits = logits4[:, k, :]
                        negmax = sm.tile([P, 1], f32, tag="nm", name="negmax")
                        nc.vector.tensor_reduce(
                            negmax, logits, axis=AX.X, op=OP.max, negate=True
                        )
                        eexp = sm.tile([P, E], f32, tag="ee", name="eexp")
                        sumexp = sm.tile([P, 1], f32, tag="se", name="sumexp")
                        nc.scalar.activation(
                            eexp, logits, AF.Exp, bias=negmax, accum_out=sumexp
                        )
                        rinv = sm.tile([P, 1], f32, tag="ri", name="rinv")
                        nc.vector.reciprocal(rinv, sumexp)
                        nc.vector.tensor_scalar_mul(probs_all[:, tt, :], eexp, rinv)
                        tmp8 = sm.tile([P, E], f32, tag="t8", name="tmp8")
                        nc.vector.tensor_tensor(tmp8, eexp, sel_sb, OP.mult)
                        esum = sm.tile([P, 1], f32, tag="es", name="esum")
                        nc.vector.tensor_reduce(esum, tmp8, axis=AX.X, op=OP.add)
                        nc.gpsimd.tensor_tensor(
                            p_sb[:, tt : tt + 1], esum, rinv, OP.mult
                        )
                nc.sync.dma_start(
                    probs_d.rearrange("(o p) e -> p o e", p=P), probs_all
                )

            # block 0: fc1, then router, then fc2 (its epilogue needs p_sb)
            load_xtb(0)
            h0 = fc1_block(0)
            for tb in range(1, NTB):
                load_xtb(tb)
            router()
            fc2_block(0, h0)
            for tb in range(1, NTB):
                h = fc1_block(tb)
                fc2_block(tb, h)

    nc.compile()
    return nc


def _get_nc():
    if "nc" not in _CACHE:
        _CACHE["nc"] = _build_nc()
    return _CACHE["nc"]


def _make_in_maps(x, router_w, router_b, fc1_weight, fc1_bias, fc2_weight):
    import ml_dtypes

    bf16 = ml_dtypes.bfloat16
    x_flat = np.asarray(x, dtype=np.float32).reshape(T, D)
    xtb = np.ascontiguousarray(x_flat.T.astype(bf16))        # [D, T] bf16
    rw = np.ascontiguousarray(np.asarray(router_w, dtype=np.float32).astype(bf16))
    rbb = np.ascontiguousarray(
        np.tile(np.asarray(router_b, dtype=np.float32)[None, :], (P, 1))
    )
    in_maps = []
    for c in range(NCORES):
        sel = np.zeros((P, E), dtype=np.float32)
        sel[:, c] = 1.0
        w1 = np.asarray(fc1_weight[c], dtype=np.float32).astype(bf16)  # [D, H]
        # pack to [P, HK, DT, P]: w1p[p, hk, dt, j] = W1[dt*128+p, hk*128+j]
        w1p = np.ascontiguousarray(w1.reshape(DT, P, HK, P).transpose(1, 2, 0, 3))
        w2 = np.ascontiguousarray(
            np.asarray(fc2_weight[c], dtype=np.float32).astype(bf16)
        )  # [H, D]
        b1 = np.ascontiguousarray(np.asarray(fc1_bias[c], dtype=np.float32))
        in_maps.append(
            {
                "xtb": xtb,
                "rw": rw,
                "rbb": rbb,
                "sel": sel,
                "w1p": w1p,
                "b1": b1,
                "w2": w2,
            }
        )
    return in_maps


def kernel(x, router_w, router_b, fc1_weight, fc1_bias, fc2_weight, fc2_bias):
    from concourse import bass_utils

    nc = _get_nc()
    in_maps = _make_in_maps(x, router_w, router_b, fc1_weight, fc1_bias, fc2_weight)

    trace = bool(int(os.environ.get("MOE_TRACE", "0")))
    res = bass_utils.run_bass_kernel_spmd(
        nc, in_maps, core_ids=list(range(NCORES)), trace=trace
    )
    if trace:
        _CACHE["last_results"] = res
        if res.exec_time_ns is not None:
            print(f"HW exec time: {res.exec_time_ns} ns")

    y = np.zeros((T, D), dtype=np.float64)
    for c in range(NCORES):
        y += res.results[c]["yp"].astype(np.float64)
    probs = res.results[0]["probs"]
    # fc2_bias contribution of the probability-weighted combine (zeros here,
    # kept for generality): sum_e probs[:, e] * b2[e, :]
    y += probs.astype(np.float64) @ np.asarray(fc2_bias, dtype=np.float64)
    return (
        y.astype(np.float32).reshape(B, S, D),
        probs.reshape(B, S, E).astype(np.float32),
    )


# revision 22
# speedup vs baseline: 1.0142x; 1.0142x over previous
"""MoE FFN (dense routing) Trainium2 kernel — expert-parallel over 8 NeuronCores.

Reference math (T=2048 tokens, D=1024, H=4096, E=8 experts, all dense):
    logits = x @ router_w + router_b          [T, E]
    probs  = softmax(logits)                  [T, E]
    h_e    = gelu(x @ W1[e] + b1[e])          [T, H]   (exact erf gelu)
    out_e  = h_e @ W2[e] + b2[e]              [T, D]
    y      = sum_e probs[:, e] * out_e        [T, D]
    returns (y, probs)

Sharding: expert parallel — core c owns expert e=c, computes the full dense
token set through its expert, scales by probs[:, e] on-chip, and the partial
outputs are summed on the host (the "all-reduce of the probability-weighted
combine"). Every core computes the router; core 0's probs are returned.
fc2_bias (zeros in this problem) is folded in on the host as probs @ fc2_bias.

Host-side input prep (free, not on the HW critical path): x is laid out as
x_T [D, T] bf16 (fc1 moving operand + router); W1 is packed [P, HK, DT, P]
bf16 for single-descriptor-per-partition DMA; W2 cast to bf16.

On-chip dataflow per core:
    fc1: h_T[hk] [128,512] = W1_chunk.T @ x_Tb  (bf16, PSUM fp32 accum)
         gelu+bias fused on ScalarE -> h bf16
    router (after fc1 of block 0): logits_T [8,T] via bf16 matmuls
         (N=512, full PE rate), PE-transposed back to [T,8] tiles,
         softmax on ACT/DVE, expert column via one-hot dot
    fc2: y[mt,dc] [128,512] accumulates over 32 H-chunks in 8 PSUM banks
         epilogue: ScalarE copy with per-partition scale = probs[:, e]
"""

import os
import sys

import numpy as np

for _p in ("/root/.axon_site/_ro/trn_rl_repo", "/opt/trn_rl_repo"):
    if os.path.isdir(_p) and _p not in sys.path:
        sys.path.append(_p)

B, S, D, H, E = 2, 1024, 1024, 4096, 8
T = B * S          # 2048 tokens
NCORES = 8
P = 128            # partitions
TT = T // P        # 16 token tiles of 128
NTB = 4            # token blocks for the FFN phase
TBLK = T // NTB    # 512 tokens per block
DT = D // P        # 8 contraction chunks for fc1 / router
HK = H // P        # 32 H-chunks
NMT = TBLK // P    # 4 M-subtiles per block
NDC = D // 512     # 2 N-subtiles of 512 per block

_CACHE = {}


def _build_nc():
    import concourse.bass as bass  # noqa: F401
    import concourse.tile as tile
    from concourse import bacc, mybir
    from concourse.masks import make_identity

    f32 = mybir.dt.float32
    bf16 = mybir.dt.bfloat16
    AF = mybir.ActivationFunctionType
    OP = mybir.AluOpType
    AX = mybir.AxisListType

    nc = bacc.Bacc(
        "TRN2",
        target_bir_lowering=False,
        debug=False,
        enable_asserts=False,
        num_devices=NCORES,
    )

    xtb_d = nc.dram_tensor("xtb", [D, T], bf16, kind="ExternalInput").ap()
    rw_d = nc.dram_tensor("rw", [D, E], bf16, kind="ExternalInput").ap()
    rbb_d = nc.dram_tensor("rbb", [P, E], f32, kind="ExternalInput").ap()
    sel_d = nc.dram_tensor("sel", [P, E], f32, kind="ExternalInput").ap()
    w1_d = nc.dram_tensor("w1p", [P, HK, DT, P], bf16, kind="ExternalInput").ap()
    b1_d = nc.dram_tensor("b1", [H], f32, kind="ExternalInput").ap()
    w2_d = nc.dram_tensor("w2", [H, D], bf16, kind="ExternalInput").ap()
    y_d = nc.dram_tensor("yp", [T, D], f32, kind="ExternalOutput").ap()
    probs_d = nc.dram_tensor("probs", [T, E], f32, kind="ExternalOutput").ap()

    with tile.TileContext(nc) as tc:
        from contextlib import ExitStack

        ctx = ExitStack()
        with ctx:
            consts = ctx.enter_context(tc.tile_pool(name="consts", bufs=1))
            xtbp = ctx.enter_context(tc.tile_pool(name="xtbp", bufs=1))
            w1p = ctx.enter_context(tc.tile_pool(name="w1p", bufs=8))
            w2p = ctx.enter_context(tc.tile_pool(name="w2p", bufs=8))
            hp = ctx.enter_context(tc.tile_pool(name="hp", bufs=HK))
            yp = ctx.enter_context(tc.tile_pool(name="yp", bufs=6))
            sm = ctx.enter_context(tc.tile_pool(name="sm", bufs=3))
            psp = ctx.enter_context(tc.tile_pool(name="psp", bufs=8, space="PSUM"))

            # constants on the SWDGE (gpsimd) queue so they don't head-block
            # the bulk sync-queue streams
            ident = consts.tile([P, P], f32, tag="ident", name="ident")
            make_identity(nc, ident)
            rw_sb = consts.tile([P, DT, E], bf16, tag="rw", name="rw_sb")
            nc.gpsimd.dma_start(rw_sb, rw_d.rearrange("(o p) e -> p o e", p=P))
            rbb_sb = consts.tile([P, E], f32, tag="rbb", name="rbb_sb")
            nc.gpsimd.dma_start(rbb_sb, rbb_d)
            sel_sb = consts.tile([P, E], f32, tag="sel", name="sel_sb")
            nc.gpsimd.dma_start(sel_sb, sel_d)
            b1_sb = consts.tile([P, HK], f32, tag="b1", name="b1_sb")
            nc.gpsimd.dma_start(b1_sb, b1_d.rearrange("(o p) -> p o", p=P))
            p_sb = consts.tile([P, TT], f32, tag="psb", name="p_sb")
            probs_all = consts.tile([P, TT, E], f32, tag="probs", name="probs_all")

            # x_T bf16 as one [P, DT, T] tile (p,dt,t) = x_T[dt*128+p, t];
            # block-0 columns DMA'd first so fc1(0) starts immediately, the
            # rest before the router (which reads all of T)
            xTb = xtbp.tile([P, DT, T], bf16, name="xTb")
            xtb_r = xtb_d.rearrange("(dt p) t -> p dt t", p=P)

            def load_xtb(tb):
                cs = slice(tb * TBLK, (tb + 1) * TBLK)
                nc.sync.dma_start(xTb[:, :, cs], xtb_r[:, :, cs])
            w2_r = w2_d.rearrange("(o p) d -> p o d", p=P)

            def fc1_block(tb):
                cs = slice(tb * TBLK, (tb + 1) * TBLK)
                h_tiles = []
                for hk2 in range(HK // 2):
                    w1_t = w1p.tile([P, 2, DT, P], bf16, name="w1_t")
                    nc.sync.dma_start(w1_t, w1_d[:, 2 * hk2 : 2 * hk2 + 2])
                    for j in range(2):
                        hk = 2 * hk2 + j
                        ps_h = psp.tile([P, TBLK], f32, tag="ps", name="ps_h")
                        for dt in range(DT):
                            nc.tensor.matmul(
                                ps_h,
                                lhsT=w1_t[:, j, dt, :],
                                rhs=xTb[:, dt, cs],
                                start=(dt == 0),
                                stop=(dt == DT - 1),
                            )
                        h_t = hp.tile([P, TBLK], bf16, name="h_t")
                        nc.scalar.activation(
                            h_t, ps_h, AF.Gelu, bias=b1_sb[:, hk : hk + 1]
                        )
                        h_tiles.append(h_t)
                return h_tiles

            def fc2_block(tb, h_tiles):
                ps_y = [
                    psp.tile([P, 512], f32, tag="ps", name=f"ps_y{g}")
                    for g in range(NMT * NDC)
                ]
                for hk2 in range(HK // 2):
                    w2_t = w2p.tile([P, 2, D], bf16, name="w2_t")
                    nc.sync.dma_start(w2_t, w2_r[:, 2 * hk2 : 2 * hk2 + 2])
                    for j in range(2):
                        hk = 2 * hk2 + j
                        for mt in range(NMT):
                            for dc in range(NDC):
                                nc.tensor.matmul(
                                    ps_y[mt * NDC + dc],
                                    lhsT=h_tiles[hk][:, mt * P : (mt + 1) * P],
                                    rhs=w2_t[:, j, dc * 512 : (dc + 1) * 512],
                                    start=(hk == 0),
                                    stop=(hk == HK - 1),
                                )
                for mt in range(NMT):
                    y_t = yp.tile([P, D], f32, name="y_t")
                    p_ap = p_sb[:, tb * NMT + mt : tb * NMT + mt + 1]
                    nc.scalar.activation(
                        y_t[:, 0:512], ps_y[mt * NDC], AF.Copy, scale=p_ap
                    )
                    nc.vector.tensor_scalar_mul(
                        y_t[:, 512:1024], ps_y[mt * NDC + 1], p_ap
                    )
                    r0 = tb * TBLK + mt * P
                    nc.sync.dma_start(y_d[r0 : r0 + P, :], y_t)

            def router():
                # logits_T [E, T] via bf16 matmuls at full PE rate (N=512).
                # All 4 matmul groups run back-to-back on PE (DVE copies
                # drain behind them), then all transposes: avoids PE<->DVE
                # round-trips per chunk.
                lt_ts = []
                for tc4 in range(NTB):
                    cs = slice(tc4 * TBLK, (tc4 + 1) * TBLK)
                    ps_lt = psp.tile([E, TBLK], f32, tag="ps", name="ps_lt")
                    for dt in range(DT):
                        nc.tensor.matmul(
                            ps_lt,
                            lhsT=rw_sb[:, dt, :],
                            rhs=xTb[:, dt, cs],
                            start=(dt == 0),
                            stop=(dt == DT - 1),
                        )
                    lt_t = sm.tile([E, TBLK], f32, tag="lt", name="lt_t", bufs=4)
                    nc.vector.tensor_copy(lt_t, ps_lt)
                    lt_ts.append(lt_t)
                for tc4 in range(NTB):
                    ps_lg = psp.tile([P, TBLK // P, E], f32, tag="ps", name="ps_lg")
                    for k in range(TBLK // P):
                        nc.tensor.transpose(
                            ps_lg[:, k, :],
                            lt_ts[tc4][:, k * P : (k + 1) * P],
                            ident[:E, :E],
                        )
                    logits4 = sm.tile([P, TBLK // P, E], f32, tag="lg4", name="logits4")
                    nc.vector.tensor_tensor(
                        logits4, ps_lg, rbb_sb[:, None, :].to_broadcast(
                            (P, TBLK // P, E)), OP.add
                    )
                    for k in range(TBLK // P):
                        tt = tc4 * (TBLK // P) + k
                        logits = logits4[:, k, :]
                        negmax = sm.tile([P, 1], f32, tag="nm", name="negmax")
                        nc.vector.tensor_reduce(
                            negmax, logits, axis=AX.X, op=OP.max, negate=True
                        )
                        eexp = sm.tile([P, E], f32, tag="ee", name="eexp")
                        sumexp = sm.tile([P, 1], f32, tag="se", name="sumexp")
                        nc.scalar.activation(
                            eexp, logits, AF.Exp, bias=negmax, accum_out=sumexp
                        )
                        rinv = sm.tile([P, 1], f32, tag="ri", name="rinv")
                        nc.vector.reciprocal(rinv, sumexp)
                        nc.vector.tensor_scalar_mul(probs_all[:, tt, :], eexp, rinv)
                        tmp8 = sm.tile([P, E], f32, tag="t8", name="tmp8")
                        nc.vector.tensor_tensor(tmp8, eexp, sel_sb, OP.mult)
                        esum = sm.tile([P, 1], f32, tag="es", name="esum")
                        nc.vector.tensor_reduce(esum, tmp8, axis=AX.X, op=OP.add)
                        nc.gpsimd.tensor_tensor(
                            p_sb[:, tt : tt + 1], esum, rinv, OP.mult
                        )
                nc.sync.dma_start(
                    probs_d.rearrange("(o p) e -> p o e", p=P), probs_all
                )

            # block 0: fc1, then router, then fc2 (its epilogue needs p_sb)
            load_xtb(0)
            h0 = fc1_block(0)
            for tb in range(1, NTB):
                load_xtb(tb)
            router()
            fc2_block(0, h0)
            for tb in range(1, NTB):
                h = fc1_block(tb)
                fc2_block(tb, h)

    nc.compile()
    return nc


def _get_nc():
    if "nc" not in _CACHE:
        _CACHE["nc"] = _build_nc()
    return _CACHE["nc"]


def _make_in_maps(x, router_w, router_b, fc1_weight, fc1_bias, fc2_weight):
    import ml_dtypes

    bf16 = ml_dtypes.bfloat16
    x_flat = np.asarray(x, dtype=np.float32).reshape(T, D)
    xtb = np.ascontiguousarray(x_flat.T.astype(bf16))        # [D, T] bf16
    rw = np.ascontiguousarray(np.asarray(router_w, dtype=np.float32).astype(bf16))
    rbb = np.ascontiguousarray(
        np.tile(np.asarray(router_b, dtype=np.float32)[None, :], (P, 1))
    )
    in_maps = []
    for c in range(NCORES):
        sel = np.zeros((P, E), dtype=np.float32)
        sel[:, c] = 1.0
        w1 = np.asarray(fc1_weight[c], dtype=np.float32).astype(bf16)  # [D, H]
        # pack to [P, HK, DT, P]: w1p[p, hk, dt, j] = W1[dt*128+p, hk*128+j]
        w1p = np.ascontiguousarray(w1.reshape(DT, P, HK, P).transpose(1, 2, 0, 3))
        w2 = np.ascontiguousarray(
            np.asarray(fc2_weight[c], dtype=np.float32).astype(bf16)
        )  # [H, D]
        b1 = np.ascontiguousarray(np.asarray(fc1_bias[c], dtype=np.float32))
        in_maps.append(
            {
                "xtb": xtb,
                "rw": rw,
                "rbb": rbb,
                "sel": sel,
                "w1p": w1p,
                "b1": b1,
                "w2": w2,
            }
        )
    return in_maps


def kernel(x, router_w, router_b, fc1_weight, fc1_bias, fc2_weight, fc2_bias):
    from concourse import bass_utils

    nc = _get_nc()
    in_maps = _make_in_maps(x, router_w, router_b, fc1_weight, fc1_bias, fc2_weight)

    trace = bool(int(os.environ.get("MOE_TRACE", "0")))
    res = bass_utils.run_bass_kernel_spmd(
        nc, in_maps, core_ids=list(range(NCORES)), trace=trace
    )
    if trace:
        _CACHE["last_results"] = res
        if res.exec_time_ns is not None:
            print(f"HW exec time: {res.exec_time_ns} ns")

    y = np.zeros((T, D), dtype=np.float64)
    for c in range(NCORES):
        y += res.results[c]["yp"].astype(np.float64)
    probs = res.results[0]["probs"]
    # fc2_bias contribution of the probability-weighted combine (zeros here,
    # kept for generality): sum_e probs[:, e] * b2[e, :]
    y += probs.astype(np.float64) @ np.asarray(fc2_bias, dtype=np.float64)
    return (
        y.astype(np.float32).reshape(B, S, D),
        probs.reshape(B, S, E).astype(np.float32),
    )


# revision 23
# speedup vs baseline: 1.0154x; 1.0012x over previous
"""MoE FFN (dense routing) Trainium2 kernel — expert-parallel over 8 NeuronCores.

Reference math (T=2048 tokens, D=1024, H=4096, E=8 experts, all dense):
    logits = x @ router_w + router_b          [T, E]
    probs  = softmax(logits)                  [T, E]
    h_e    = gelu(x @ W1[e] + b1[e])          [T, H]   (exact erf gelu)
    out_e  = h_e @ W2[e] + b2[e]              [T, D]
    y      = sum_e probs[:, e] * out_e        [T, D]
    returns (y, probs)

Sharding: expert parallel — core c owns expert e=c, computes the full dense
token set through its expert, scales by probs[:, e] on-chip, and the partial
outputs are summed on the host (the "all-reduce of the probability-weighted
combine"). Every core computes the router; core 0's probs are returned.
fc2_bias (zeros in this problem) is folded in on the host as probs @ fc2_bias.

Host-side input prep (free, not on the HW critical path): x is laid out as
x_T [D, T] bf16 (fc1 moving operand + router); W1 is packed [P, HK, DT, P]
bf16 for single-descriptor-per-partition DMA; W2 cast to bf16.

On-chip dataflow per core:
    fc1: h_T[hk] [128,512] = W1_chunk.T @ x_Tb  (bf16, PSUM fp32 accum)
         gelu+bias fused on ScalarE -> h bf16
    router (after fc1 of block 0): logits_T [8,T] via bf16 matmuls
         (N=512, full PE rate), PE-transposed back to [T,8] tiles,
         softmax on ACT/DVE, expert column via one-hot dot
    fc2: y[mt,dc] [128,512] accumulates over 32 H-chunks in 8 PSUM banks
         epilogue: ScalarE copy with per-partition scale = probs[:, e]
"""

import os
import sys

import numpy as np

for _p in ("/root/.axon_site/_ro/trn_rl_repo", "/opt/trn_rl_repo"):
    if os.path.isdir(_p) and _p not in sys.path:
        sys.path.append(_p)

B, S, D, H, E = 2, 1024, 1024, 4096, 8
T = B * S          # 2048 tokens
NCORES = 8
P = 128            # partitions
TT = T // P        # 16 token tiles of 128
NTB = 4            # token blocks for the FFN phase
TBLK = T // NTB    # 512 tokens per block
DT = D // P        # 8 contraction chunks for fc1 / router
HK = H // P        # 32 H-chunks
NMT = TBLK // P    # 4 M-subtiles per block
NDC = D // 512     # 2 N-subtiles of 512 per block

_CACHE = {}


def _build_nc():
    import concourse.bass as bass  # noqa: F401
    import concourse.tile as tile
    from concourse import bacc, mybir
    from concourse.masks import make_identity

    f32 = mybir.dt.float32
    bf16 = mybir.dt.bfloat16
    AF = mybir.ActivationFunctionType
    OP = mybir.AluOpType
    AX = mybir.AxisListType

    nc = bacc.Bacc(
        "TRN2",
        target_bir_lowering=False,
        debug=False,
        enable_asserts=False,
        num_devices=NCORES,
    )

    xtb_d = nc.dram_tensor("xtb", [D, T], bf16, kind="ExternalInput").ap()
    rw_d = nc.dram_tensor("rw", [D, E], bf16, kind="ExternalInput").ap()
    rbb_d = nc.dram_tensor("rbb", [P, E], f32, kind="ExternalInput").ap()
    sel_d = nc.dram_tensor("sel", [P, E], f32, kind="ExternalInput").ap()
    w1_d = nc.dram_tensor("w1p", [P, HK, DT, P], bf16, kind="ExternalInput").ap()
    b1_d = nc.dram_tensor("b1", [H], f32, kind="ExternalInput").ap()
    w2_d = nc.dram_tensor("w2", [H, D], bf16, kind="ExternalInput").ap()
    y_d = nc.dram_tensor("yp", [T, D], f32, kind="ExternalOutput").ap()
    probs_d = nc.dram_tensor("probs", [T, E], f32, kind="ExternalOutput").ap()

    with tile.TileContext(nc) as tc:
        from contextlib import ExitStack

        ctx = ExitStack()
        with ctx:
            consts = ctx.enter_context(tc.tile_pool(name="consts", bufs=1))
            xtbp = ctx.enter_context(tc.tile_pool(name="xtbp", bufs=1))
            w1p = ctx.enter_context(tc.tile_pool(name="w1p", bufs=8))
            w2p = ctx.enter_context(tc.tile_pool(name="w2p", bufs=8))
            hp = ctx.enter_context(tc.tile_pool(name="hp", bufs=HK))
            yp = ctx.enter_context(tc.tile_pool(name="yp", bufs=6))
            sm = ctx.enter_context(tc.tile_pool(name="sm", bufs=3))
            psp = ctx.enter_context(tc.tile_pool(name="psp", bufs=8, space="PSUM"))

            # constants on the SWDGE (gpsimd) queue so they don't head-block
            # the bulk sync-queue streams
            ident = consts.tile([P, P], f32, tag="ident", name="ident")
            make_identity(nc, ident)
            rw_sb = consts.tile([P, DT, E], bf16, tag="rw", name="rw_sb")
            nc.gpsimd.dma_start(rw_sb, rw_d.rearrange("(o p) e -> p o e", p=P))
            rbb_sb = consts.tile([P, E], f32, tag="rbb", name="rbb_sb")
            nc.gpsimd.dma_start(rbb_sb, rbb_d)
            sel_sb = consts.tile([P, E], f32, tag="sel", name="sel_sb")
            nc.gpsimd.dma_start(sel_sb, sel_d)
            b1_sb = consts.tile([P, HK], f32, tag="b1", name="b1_sb")
            nc.gpsimd.dma_start(b1_sb, b1_d.rearrange("(o p) -> p o", p=P))
            p_sb = consts.tile([P, TT], f32, tag="psb", name="p_sb")
            probs_all = consts.tile([P, TT, E], f32, tag="probs", name="probs_all")

            # x_T bf16 as one [P, DT, T] tile (p,dt,t) = x_T[dt*128+p, t];
            # block-0 columns DMA'd first so fc1(0) starts immediately, the
            # rest before the router (which reads all of T)
            xTb = xtbp.tile([P, DT, T], bf16, name="xTb")
            xtb_r = xtb_d.rearrange("(dt p) t -> p dt t", p=P)

            def load_xtb(tb):
                cs = slice(tb * TBLK, (tb + 1) * TBLK)
                if tb == 0:
                    # split so the first fc1 matmuls (dt 0-3) start after
                    # half the transfer
                    nc.sync.dma_start(xTb[:, 0:4, cs], xtb_r[:, 0:4, cs])
                    nc.sync.dma_start(xTb[:, 4:8, cs], xtb_r[:, 4:8, cs])
                else:
                    nc.sync.dma_start(xTb[:, :, cs], xtb_r[:, :, cs])
            w2_r = w2_d.rearrange("(o p) d -> p o d", p=P)

            def fc1_block(tb):
                cs = slice(tb * TBLK, (tb + 1) * TBLK)
                h_tiles = []
                for hk2 in range(HK // 2):
                    w1_t = w1p.tile([P, 2, DT, P], bf16, name="w1_t")
                    nc.sync.dma_start(w1_t, w1_d[:, 2 * hk2 : 2 * hk2 + 2])
                    for j in range(2):
                        hk = 2 * hk2 + j
                        ps_h = psp.tile([P, TBLK], f32, tag="ps", name="ps_h")
                        for dt in range(DT):
                            nc.tensor.matmul(
                                ps_h,
                                lhsT=w1_t[:, j, dt, :],
                                rhs=xTb[:, dt, cs],
                                start=(dt == 0),
                                stop=(dt == DT - 1),
                            )
                        h_t = hp.tile([P, TBLK], bf16, name="h_t")
                        nc.scalar.activation(
                            h_t, ps_h, AF.Gelu, bias=b1_sb[:, hk : hk + 1]
                        )
                        h_tiles.append(h_t)
                return h_tiles

            def fc2_block(tb, h_tiles):
                ps_y = [
                    psp.tile([P, 512], f32, tag="ps", name=f"ps_y{g}")
                    for g in range(NMT * NDC)
                ]
                for hk2 in range(HK // 2):
                    w2_t = w2p.tile([P, 2, D], bf16, name="w2_t")
                    nc.sync.dma_start(w2_t, w2_r[:, 2 * hk2 : 2 * hk2 + 2])
                    for j in range(2):
                        hk = 2 * hk2 + j
                        for mt in range(NMT):
                            for dc in range(NDC):
                                nc.tensor.matmul(
                                    ps_y[mt * NDC + dc],
                                    lhsT=h_tiles[hk][:, mt * P : (mt + 1) * P],
                                    rhs=w2_t[:, j, dc * 512 : (dc + 1) * 512],
                                    start=(hk == 0),
                                    stop=(hk == HK - 1),
                                )
                for mt in range(NMT):
                    y_t = yp.tile([P, D], f32, name="y_t")
                    p_ap = p_sb[:, tb * NMT + mt : tb * NMT + mt + 1]
                    nc.scalar.activation(
                        y_t[:, 0:512], ps_y[mt * NDC], AF.Copy, scale=p_ap
                    )
                    nc.vector.tensor_scalar_mul(
                        y_t[:, 512:1024], ps_y[mt * NDC + 1], p_ap
                    )
                    r0 = tb * TBLK + mt * P
                    nc.sync.dma_start(y_d[r0 : r0 + P, :], y_t)

            def router():
                # logits_T [E, T] via bf16 matmuls at full PE rate (N=512).
                # All 4 matmul groups run back-to-back on PE (DVE copies
                # drain behind them), then all transposes: avoids PE<->DVE
                # round-trips per chunk.
                lt_ts = []
                for tc4 in range(NTB):
                    cs = slice(tc4 * TBLK, (tc4 + 1) * TBLK)
                    ps_lt = psp.tile([E, TBLK], f32, tag="ps", name="ps_lt")
                    for dt in range(DT):
                        nc.tensor.matmul(
                            ps_lt,
                            lhsT=rw_sb[:, dt, :],
                            rhs=xTb[:, dt, cs],
                            start=(dt == 0),
                            stop=(dt == DT - 1),
                        )
                    lt_t = sm.tile([E, TBLK], f32, tag="lt", name="lt_t", bufs=4)
                    nc.vector.tensor_copy(lt_t, ps_lt)
                    lt_ts.append(lt_t)
                for tc4 in range(NTB):
                    ps_lg = psp.tile([P, TBLK // P, E], f32, tag="ps", name="ps_lg")
                    for k in range(TBLK // P):
                        nc.tensor.transpose(
                            ps_lg[:, k, :],
                            lt_ts[tc4][:, k * P : (k + 1) * P],
                            ident[:E, :E],
                        )
                    logits4 = sm.tile([P, TBLK // P, E], f32, tag="lg4", name="logits4")
                    nc.vector.tensor_tensor(
                        logits4, ps_lg, rbb_sb[:, None, :].to_broadcast(
                            (P, TBLK // P, E)), OP.add
                    )
                    for k in range(TBLK // P):
                        tt = tc4 * (TBLK // P) + k
                        logits = logits4[:, k, :]
                        negmax = sm.tile([P, 1], f32, tag="nm", name="negmax")
                        nc.vector.tensor_reduce(
                            negmax, logits, axis=AX.X, op=OP.max, negate=True
                        )
                        eexp = sm.tile([P, E], f32, tag="ee", name="eexp")
                        sumexp = sm.tile([P, 1], f32, tag="se", name="sumexp")
                        nc.scalar.activation(
                            eexp, logits, AF.Exp, bias=negmax, accum_out=sumexp
                        )
                        rinv = sm.tile([P, 1], f32, tag="ri", name="rinv")
                        nc.vector.reciprocal(rinv, sumexp)
                        nc.vector.tensor_scalar_mul(probs_all[:, tt, :], eexp, rinv)
                        tmp8 = sm.tile([P, E], f32, tag="t8", name="tmp8")
                        nc.vector.tensor_tensor(tmp8, eexp, sel_sb, OP.mult)
                        esum = sm.tile([P, 1], f32, tag="es", name="esum")
                        nc.vector.tensor_reduce(esum, tmp8, axis=AX.X, op=OP.add)
                        nc.gpsimd.tensor_tensor(
                            p_sb[:, tt : tt + 1], esum, rinv, OP.mult
                        )
                nc.sync.dma_start(
                    probs_d.rearrange("(o p) e -> p o e", p=P), probs_all
                )

            # block 0: fc1, then router, then fc2 (its epilogue needs p_sb)
            load_xtb(0)
            h0 = fc1_block(0)
            for tb in range(1, NTB):
                load_xtb(tb)
            router()
            fc2_block(0, h0)
            for tb in range(1, NTB):
                h = fc1_block(tb)
                fc2_block(tb, h)

    nc.compile()
    return nc


def _get_nc():
    if "nc" not in _CACHE:
        _CACHE["nc"] = _build_nc()
    return _CACHE["nc"]


def _make_in_maps(x, router_w, router_b, fc1_weight, fc1_bias, fc2_weight):
    import ml_dtypes

    bf16 = ml_dtypes.bfloat16
    x_flat = np.asarray(x, dtype=np.float32).reshape(T, D)
    xtb = np.ascontiguousarray(x_flat.T.astype(bf16))        # [D, T] bf16
    rw = np.ascontiguousarray(np.asarray(router_w, dtype=np.float32).astype(bf16))
    rbb = np.ascontiguousarray(
        np.tile(np.asarray(router_b, dtype=np.float32)[None, :], (P, 1))
    )
    in_maps = []
    for c in range(NCORES):
        sel = np.zeros((P, E), dtype=np.float32)
        sel[:, c] = 1.0
        w1 = np.asarray(fc1_weight[c], dtype=np.float32).astype(bf16)  # [D, H]
        # pack to [P, HK, DT, P]: w1p[p, hk, dt, j] = W1[dt*128+p, hk*128+j]
        w1p = np.ascontiguousarray(w1.reshape(DT, P, HK, P).transpose(1, 2, 0, 3))
        w2 = np.ascontiguousarray(
            np.asarray(fc2_weight[c], dtype=np.float32).astype(bf16)
        )  # [H, D]
        b1 = np.ascontiguousarray(np.asarray(fc1_bias[c], dtype=np.float32))
        in_maps.append(
            {
                "xtb": xtb,
                "rw": rw,
                "rbb": rbb,
                "sel": sel,
                "w1p": w1p,
                "b1": b1,
                "w2": w2,
            }
        )
    return in_maps


def kernel(x, router_w, router_b, fc1_weight, fc1_bias, fc2_weight, fc2_bias):
    from concourse import bass_utils

    nc = _get_nc()
    in_maps = _make_in_maps(x, router_w, router_b, fc1_weight, fc1_bias, fc2_weight)

    trace = bool(int(os.environ.get("MOE_TRACE", "0")))
    res = bass_utils.run_bass_kernel_spmd(
        nc, in_maps, core_ids=list(range(NCORES)), trace=trace
    )
    if trace:
        _CACHE["last_results"] = res
        if res.exec_time_ns is not None:
            print(f"HW exec time: {res.exec_time_ns} ns")

    y = np.zeros((T, D), dtype=np.float64)
    for c in range(NCORES):
        y += res.results[c]["yp"].astype(np.float64)
    probs = res.results[0]["probs"]
    # fc2_bias contribution of the probability-weighted combine (zeros here,
    # kept for generality): sum_e probs[:, e] * b2[e, :]
    y += probs.astype(np.float64) @ np.asarray(fc2_bias, dtype=np.float64)
    return (
        y.astype(np.float32).reshape(B, S, D),
        probs.reshape(B, S, E).astype(np.float32),
    )
